# revision 34
# baseline (speedup 1.0000x reference)
"""Fused LayerNorm + multi-head attention + output projection for TRN2.

Sharding over 8 NeuronCores: core c handles batch c//2 and head-half c%2
(8 of 16 heads). Head-parallel QKV/attention, row-parallel proj; the
cross-core reduction of proj partials happens on the host during unshard
(pairs of cores share a batch).

Schedule: attention runs as 16 pair-chains (4 i-quarters x 4
head-pairs); the two heads of a pair occupy PE row-halves
(tile_position packing) so their 64-contraction S matmuls run
concurrently. Each j-step produces one [128,1024] PSUM tile (both
heads' logits side by side) consumed by a single exp ACTIVATE. PSUM:
4 banks S double-buffer + 2 banks O accumulators + 2 spare banks
through which qk projection bursts and the output projection are
threaded as PE filler between chain steps (next-chain qk requirements
prioritized, softmax-normalize multiplies deferred past the 1/l
broadcast DMA roundtrip, deep pT pool to ride out scheduler
reordering). The PE is the saturated engine (~370us busy of ~455).

Device layout notes:
  - LayerNorm gamma/beta, the attention scale and log2(e) are folded into
    w_qkv / b_qkv on the host; exp(s) becomes exp(ln2 * s') on ACT's free
    affine. The v bias is folded into the host-side proj bias
    (O/l has +bv exactly, since sum(p)/l == 1).
  - q,k are produced transposed ([dims, tokens]); v token-major with an
    appended ones column so the P.V matmul also yields the softmax
    denominator (row 64 of the O accumulator).
  - exp() runs without max-subtraction: logits are ~N(0,1), fp32 exp
    cannot overflow here.
  - LN normalize + the xnT eviction run on the Scalar engine (idle during
    phase 1); stats stay on DVE.
"""

import sys

sys.path.insert(0, "/opt/trn_rl_repo")

import math

import numpy as np
import ml_dtypes

N = 2048          # tokens per batch
D = 1024          # model dim
HL = 8            # heads per core
DH = 64           # head dim
INNER_L = HL * DH  # 512, per-core inner width
TT = N // 128     # 16 token tiles
KC = D // 128     # 8 dim chunks
SCALE = DH ** -0.5
LOG2E = math.log2(math.e)
LN2 = math.log(2.0)

BF16 = ml_dtypes.bfloat16

_CACHE = {}


def _build_nc():
    import concourse.bass as bass
    import concourse.mybir as mybir
    import concourse.tile as tile
    from concourse import bacc

    F32 = mybir.dt.float32
    BF = mybir.dt.bfloat16
    AF = mybir.ActivationFunctionType
    OP = mybir.AluOpType

    nc = bacc.Bacc("TRN2", target_bir_lowering=False)

    x_in = nc.declare_dram_parameter("x", [N, D], F32, isOutput=False)
    wqkv_in = nc.declare_dram_parameter("wqkv", [D, 3 * INNER_L], BF, isOutput=False)
    bqk_in = nc.declare_dram_parameter("bqk", [128, 8], F32, isOutput=False)
    wproj_in = nc.declare_dram_parameter("wproj", [INNER_L, D], BF, isOutput=False)
    ident_in = nc.declare_dram_parameter("ident", [128, 128], F32, isOutput=False)
    out_ext = nc.declare_dram_parameter("out", [N, D], F32, isOutput=True)

    with tile.TileContext(nc) as tc:
        with (
            tc.tile_pool(name="persist", bufs=1) as persist,
            tc.tile_pool(name="xload", bufs=2) as xload,
            tc.tile_pool(name="xnorm", bufs=2) as xnorm,
            tc.tile_pool(name="lnstat", bufs=8) as lnstat,
            tc.tile_pool(name="ptile", bufs=8) as ptile,
            tc.tile_pool(name="lrow", bufs=4) as lrow,
            tc.tile_pool(name="outsb", bufs=2) as outsb,
            tc.tile_pool(name="ldram", bufs=4, space="DRAM") as ldram,
            tc.tile_pool(name="ps_st", bufs=2, space="PSUM") as ps_st,
            tc.tile_pool(name="ps_po", bufs=2, space="PSUM") as ps_po,
            tc.tile_pool(name="ps_aux", bufs=2, space="PSUM") as ps_aux,
        ):
            # ---- persistent tiles ----
            w_sb = persist.tile([128, KC, 1536], BF, tag="w_sb")
            wproj_sb = persist.tile([128, 4, D], BF, tag="wproj_sb")
            bqk_sb = persist.tile([128, 8], F32, tag="bqk_sb")
            ident = persist.tile([128, 128], F32, tag="ident")
            eps_t = persist.tile([128, 1], F32, tag="eps_t")
            xnT = persist.tile([128, KC, N], BF, tag="xnT")
            qkT = persist.tile([128, 8, N], BF, tag="qkT")
            v_all = persist.tile([128, TT, HL, DH + 1], BF, tag="v_all")
            ocatT = persist.tile([128, 4, N], BF, tag="ocatT")

            # v_all ones-fill on GpSimd (idle engine): the 16KB/partition
            # memset would stall the DVE right when LayerNorm stats need it.
            nc.gpsimd.memset(v_all, 1.0)
            # ident: the first PE transpose gates on it. Route through a
            # DVE copy so the transpose needs only one wait proc (PE
            # instructions have a tight HW wait-slot budget).
            ident_raw = persist.tile([128, 128], F32, tag="ident_raw")
            nc.sync.dma_start(out=ident_raw, in_=ident_in[:, :])
            nc.vector.tensor_copy(out=ident, in_=ident_raw)
            nc.sync.dma_start(out=bqk_sb, in_=bqk_in[:, :])
            # v columns first so the phase-1 v matmuls unblock early
            nc.sync.dma_start(
                out=w_sb[:, :, 1024:1536],
                in_=wqkv_in[:, 1024:1536].rearrange("(c p) d -> p c d", p=128),
            )
            nc.sync.dma_start(
                out=w_sb[:, :, 0:1024],
                in_=wqkv_in[:, 0:1024].rearrange("(c p) d -> p c d", p=128),
            )
            nc.sync.dma_start(
                out=wproj_sb, in_=wproj_in[:, :].rearrange("(c p) d -> p c d", p=128)
            )
            nc.vector.memset(eps_t, 1e-5)

            # ---- phase 1: LayerNorm (stats on DVE, normalize+evict on ACT)
            #      + transpose into xnT + v matmuls ----
            for tq in range(TT // 2):
                xb = xload.tile([128, 2, D], F32, tag="xb")
                xdma = nc.gpsimd if tq % 2 == 0 else nc.sync
                xdma.dma_start(
                    out=xb,
                    in_=x_in[tq * 256:(tq + 1) * 256, :].rearrange("(c p) d -> p c d", p=128),
                )
                for c in range(2):
                    t = tq * 2 + c
                    xt = xb[:, c, :]
                    stats = lnstat.tile([128, 2, 6], F32, tag="stats")
                    nc.vector.bn_stats(out=stats[:, 0, :], in_=xt[:, 0:512])
                    nc.vector.bn_stats(out=stats[:, 1, :], in_=xt[:, 512:1024])
                    mv = lnstat.tile([128, 2], F32, tag="mv")
                    nc.vector.bn_aggr(out=mv, in_=stats)
                    # mv[:,0]=mean, mv[:,1]=var -> std -> rstd; nmr = -mean*rstd
                    nc.scalar.activation(out=mv[:, 1:2], in_=mv[:, 1:2], func=AF.Sqrt, bias=eps_t)
                    rstd = lnstat.tile([128, 1], F32, tag="rstd")
                    nc.vector.reciprocal(out=rstd, in_=mv[:, 1:2])
                    nmr = lnstat.tile([128, 1], F32, tag="nmr")
                    nc.vector.tensor_scalar(
                        out=nmr, in0=mv[:, 0:1], scalar1=rstd, scalar2=-1.0,
                        op0=OP.mult, op1=OP.mult,
                    )
                    # normalize on ACT: xn = (x - mu) * rstd = x*rstd + nmr
                    xn_t = xnorm.tile([128, D], F32, tag="xn_t")
                    nc.scalar.activation(
                        out=xn_t, in_=xt, func=AF.Identity, bias=nmr, scale=rstd,
                    )
                    ptr = ps_st.tile([128, D], F32, tag="pst")
                    for kc in range(KC):
                        nc.tensor.transpose(
                            out=ptr[:, kc * 128:(kc + 1) * 128],
                            in_=xn_t[:, kc * 128:(kc + 1) * 128],
                            identity=ident,
                        )
                    # evict transposed tile on ACT (PSUM -> SBUF, fp32->bf16)
                    nc.scalar.copy(
                        out=xnT[:, :, t * 128:(t + 1) * 128],
                        in_=ptr.rearrange("p (k t) -> p k t", k=KC),
                    )
                    # v matmul for this tile right away
                    pv = ps_po.tile([128, 512], F32, tag="po")
                    for kc in range(KC):
                        nc.tensor.matmul(
                            out=pv,
                            lhsT=xnT[:, kc, t * 128:(t + 1) * 128],
                            rhs=w_sb[:, kc, 1024:1536],
                            start=(kc == 0), stop=(kc == KC - 1),
                        )
                    nc.vector.tensor_copy(
                        out=v_all[:, t, :, 0:DH],
                        in_=pv.rearrange("p (h d) -> p h d", h=HL),
                    )

            # ---- qk chunk emission (PE filler) ----
            # chunk (mt, q4): logits-space tile mt (0-3 = q dim-tiles, 4-7 = k
            # dim-tiles) for tokens [q4*512, q4*512+512).
            def emit_qk_chunk(mt, q4):
                ts = q4 * 512
                pqk = ps_aux.tile([128, 512], F32, tag="aux", name="pqk")
                for kc in range(KC):
                    nc.tensor.matmul(
                        out=pqk,
                        lhsT=w_sb[:, kc, mt * 128:(mt + 1) * 128],
                        rhs=xnT[:, kc, ts:ts + 512],
                        start=(kc == 0), stop=(kc == KC - 1),
                    )
                nc.vector.tensor_scalar(
                    out=qkT[:, mt, ts:ts + 512],
                    in0=pqk, scalar1=bqk_sb[:, mt:mt + 1], scalar2=None,
                    op0=OP.add,
                )

            # ---- proj emission (PE filler): one half-tile unit ----
            # ob staging tiles are created per token-tile-pair by the caller.
            def emit_proj_half(t, ns, ob, pool=None, ptag="aux"):
                pp = (pool or ps_aux).tile([128, 512], F32, tag=ptag, name="pp")
                for kc in range(4):
                    nc.tensor.matmul(
                        out=pp,
                        lhsT=ocatT[:, kc, t * 128:(t + 1) * 128],
                        rhs=wproj_sb[:, kc, ns * 512:(ns + 1) * 512],
                        start=(kc == 0), stop=(kc == 3),
                    )
                nc.vector.tensor_copy(out=ob[:, ns * 512:(ns + 1) * 512], in_=pp)

            # qk chunks needed before chain (iq, hq): k-chunks (4+hq, *) and
            # the q-chunk (hq, iq). They are spread through the PREVIOUS
            # chain's jc-loop as prioritized filler so neither a front-burst
            # nor a DVE-queue stall sits between chains.
            qk_emitted = set()

            def emit_qk_now(mt, q4):
                if (mt, q4) not in qk_emitted:
                    qk_emitted.add((mt, q4))
                    emit_qk_chunk(mt, q4)

            # generic (non-urgent) filler: proj halves
            filler = []
            fill_pos = 0

            def drain_filler(nmax):
                nonlocal fill_pos
                n = 0
                while fill_pos < len(filler) and n < nmax:
                    filler[fill_pos]()
                    fill_pos += 1
                    n += 1

            # proj state
            ob_tiles = {}
            proj_done = [0]

            def push_proj_for_iq(iq, alt_pool=False):
                for t in (4 * iq, 4 * iq + 1, 4 * iq + 2, 4 * iq + 3):
                    for ns in range(2):
                        # tail proj (alt_pool): odd tiles borrow the freed po
                        # pool so two proj pipelines run in parallel
                        def _p(t=t, ns=ns, alt=alt_pool):
                            if t not in ob_tiles:
                                ob_tiles[t] = outsb.tile([128, D], F32, tag="ob", name="ob")
                            if alt and t % 2 == 1:
                                emit_proj_half(t, ns, ob_tiles[t], pool=ps_po, ptag="po")
                            else:
                                emit_proj_half(t, ns, ob_tiles[t])
                            proj_done[0] += 1
                            if proj_done[0] % 2 == 0:
                                obd = ob_tiles.pop(t)
                                nc.sync.dma_start(
                                    out=out_ext[t * 128:(t + 1) * 128, :],
                                    in_=obd,
                                )
                        filler.append(_p)

            # ---- attention: 16 pair-chains ----
            chain_order = [(iq, hq) for iq in range(4) for hq in range(4)]

            def reqs_for(ci):
                if ci >= len(chain_order):
                    return []
                iq_n, hq_n = chain_order[ci]
                out = [(4 + hq_n, q4) for q4 in range(4)] + [(hq_n, iq_n)]
                return [c for c in out if c not in qk_emitted]

            # pre-emit chain 0's requirements (k tokens all, q iq=0)
            for mt, q4 in reqs_for(0):
                emit_qk_now(mt, q4)

            # deferred ocatT multiplies: the 1/l broadcast DMA roundtrip takes
            # ~3us; running the multiply at the END of its own chain blocks
            # the DVE queue (and everything behind it) at the chain boundary.
            # Instead the multiply closures run a few steps INTO the next
            # chain, by which time the roundtrip has landed.
            deferred = []

            def drain_one(ci):
                if deferred:
                    deferred.pop(0)()
                    return
                nxt = reqs_for(ci + 1)
                if nxt:
                    emit_qk_now(*nxt[0])
                    return
                drain_filler(1)

            for ci, (iq, hq) in enumerate(chain_order):
                    h_lo, h_hi = 2 * hq, 2 * hq + 1
                    # safety: any stragglers for THIS chain (normally none)
                    for mt, q4 in reqs_for(ci):
                        emit_qk_now(mt, q4)
                    po_lo = ps_po.tile([128, 512], F32, tag="po")
                    po_hi = ps_po.tile([128, 512], F32, tag="po")
                    for jc in range(TT):
                        pst = ps_st.tile([128, 1024], F32, tag="pst")
                        nc.tensor.matmul(
                            out=pst[:, 0:512],
                            lhsT=qkT[0:64, 4 + hq, jc * 128:(jc + 1) * 128],
                            rhs=qkT[0:64, hq, iq * 512:(iq + 1) * 512],
                            start=True, stop=True,
                        )
                        nc.tensor.matmul(
                            out=pst[:, 512:1024],
                            lhsT=qkT[64:128, 4 + hq, jc * 128:(jc + 1) * 128],
                            rhs=qkT[64:128, hq, iq * 512:(iq + 1) * 512],
                            start=True, stop=True,
                        )
                        pT = ptile.tile([128, 1024], BF, tag="pT")
                        nc.scalar.activation(out=pT, in_=pst, func=AF.Exp, scale=LN2)
                        nc.tensor.matmul(
                            out=po_lo[0:65, :],
                            lhsT=v_all[:, jc, h_lo, :],
                            rhs=pT[:, 0:512],
                            start=(jc == 0), stop=(jc == TT - 1),
                        )
                        nc.tensor.matmul(
                            out=po_hi[0:65, :],
                            lhsT=v_all[:, jc, h_hi, :],
                            rhs=pT[:, 512:1024],
                            start=(jc == 0), stop=(jc == TT - 1),
                        )
                        if jc in (1, 3, 5, 7, 9, 11):
                            drain_one(ci)
                    # chain end: denominators (row 64) -> 1/l, broadcast to 64
                    # partitions with a tiny K=1 PE matmul (no DRAM roundtrip),
                    # then O * (1/l) on DVE.
                    for po_x, hp in ((po_lo, 0), (po_hi, 64)):
                        lrow_s = lrow.tile([1, 512], F32, tag="lrow_s")
                        nc.vector.tensor_copy(out=lrow_s, in_=po_x[64:65, :])
                        linv = lrow.tile([1, 512], F32, tag="linv")
                        nc.vector.reciprocal_approx_fast(out=linv, in_=lrow_s)
                        linb = lrow.tile([64, 512], F32, tag="linb")
                        lb = ldram.tile([1, 512], F32, tag="lb")
                        nc.sync.dma_start(out=lb, in_=linv)
                        lb_bc = bass.AP(
                            tensor=lb.tensor, offset=lb.offset,
                            ap=[[0, 64]] + lb.ap[1:],
                        )
                        nc.sync.dma_start(out=linb, in_=lb_bc)

                        def _tt(po_x=po_x, hp=hp, hq=hq, iq=iq, linb=linb):
                            nc.vector.tensor_mul(
                                out=ocatT[hp:hp + 64, hq, iq * 512:(iq + 1) * 512],
                                in0=po_x[0:64, :], in1=linb,
                            )
                        deferred.append(_tt)
                    # after all 4 head-pairs of an i-quarter, its proj inputs
                    # are complete; queue proj fillers for the next chains.
                    if hq == 3:
                        push_proj_for_iq(iq, alt_pool=(iq == 3))

            # drain deferred multiplies and remaining filler (tail proj)
            while deferred:
                deferred.pop(0)()
            drain_filler(len(filler))

    nc.finalize()
    return nc


def _prep_in_maps(x, ln_gamma, ln_beta, w_qkv, b_qkv, w_proj):
    x = np.asarray(x, dtype=np.float32)
    ln_gamma = np.asarray(ln_gamma, dtype=np.float32)
    ln_beta = np.asarray(ln_beta, dtype=np.float32)
    w_qkv = np.asarray(w_qkv, dtype=np.float32)
    b_qkv = np.asarray(b_qkv, dtype=np.float32)
    w_proj = np.asarray(w_proj, dtype=np.float32)

    W = ln_gamma[:, None] * w_qkv          # fold gamma
    beff = b_qkv + ln_beta @ w_qkv         # fold beta
    ident = np.eye(128, dtype=np.float32)
    qs = SCALE * LOG2E                     # attention scale + exp->exp2 fold

    in_maps = []
    for c in range(8):
        b, half = divmod(c, 2)
        hs = half * INNER_L
        wq = W[:, hs:hs + INNER_L] * qs
        wk = W[:, D + hs:D + hs + INNER_L]
        wv = W[:, 2 * D + hs:2 * D + hs + INNER_L]
        bq = beff[hs:hs + INNER_L] * qs
        bk = beff[D + hs:D + hs + INNER_L]
        wqkv_c = np.ascontiguousarray(
            np.concatenate([wq, wk, wv], axis=1)
        ).astype(BF16)
        bqk_col = np.ascontiguousarray(
            np.concatenate([bq, bk]).reshape(8, 128).T
        )
        wproj_c = np.ascontiguousarray(w_proj[hs:hs + INNER_L, :]).astype(BF16)
        in_maps.append({
            "x": np.ascontiguousarray(x[b]),
            "wqkv": wqkv_c,
            "bqk": bqk_col,
            "wproj": wproj_c,
            "ident": ident,
        })
    return in_maps


def kernel(x, ln_gamma, ln_beta, w_qkv, b_qkv, w_proj, b_proj, _trace=False, _tmpdir=None):
    from concourse.bass_utils import run_bass_kernel_spmd

    if "nc" not in _CACHE:
        _CACHE["nc"] = _build_nc()
    nc = _CACHE["nc"]

    in_maps = _prep_in_maps(x, ln_gamma, ln_beta, w_qkv, b_qkv, w_proj)
    res = run_bass_kernel_spmd(
        nc, in_maps, core_ids=list(range(8)), trace=_trace, tmpdir=_tmpdir
    )
    _CACHE["last_result"] = res

    # host bias: b_proj plus the folded v-bias contribution bv @ w_proj
    b_proj = np.asarray(b_proj, dtype=np.float32)
    w_qkv64 = np.asarray(w_qkv, dtype=np.float64)
    beff = np.asarray(b_qkv, dtype=np.float64) + np.asarray(ln_beta, dtype=np.float64) @ w_qkv64
    bv_full = beff[2 * D:3 * D]
    badd = (b_proj + (bv_full @ np.asarray(w_proj, dtype=np.float64))).astype(np.float32)

    out = np.empty((4, N, D), dtype=np.float32)
    for b in range(4):
        out[b] = res.results[2 * b]["out"] + res.results[2 * b + 1]["out"] + badd
    return out
